# revision 1
# baseline (speedup 1.0000x reference)
"""MoE-routing LoRA linear for Trainium2, SPMD over 8 NeuronCores.

out = x @ base_w.T + base_b + 2.0 * lora_out, where lora_out routes each
token through its top-2 (of 8) LoRA experts with renormalized softmax gates.

Strategy: data-parallel over tokens (1024 tokens/core), weights replicated.
All heavy FLOPs are fp32r matmuls (full PE rate). The per-expert LoRA is
algebraically dense: h = x @ A_cat.T ([T,256]); gated hg = h * gates[e];
lora_out = hg @ W2 ([256,4096]) which is fused into the base matmul as two
extra contraction chunks.
"""

import numpy as np

P = 128
B, S, D, O, E, R = 4, 2048, 4096, 4096, 8, 32
T = B * S            # 8192 tokens
NCORES = 8
TC = T // NCORES     # 1024 tokens per core
TT = TC // P         # 8 token tiles per core
DC = D // P          # 32 contraction chunks for x
ER = E * R           # 256
HC = ER // P         # 2 contraction chunks for hg
KC = DC + HC         # 34 total contraction chunks in the fused matmul
ON = 512             # output tile width
OT = O // ON         # 8 output tiles
SCALING = 64.0 / 32.0

TRACE = False        # test harness sets kernel.TRACE = True for profiling
LAST_RESULT = None   # BassKernelResults of the last run (for exec_time_ns)

_compiled = None


def _build():
    import concourse.mybir as mybir
    import concourse.tile as tile
    from concourse import bacc
    from concourse.masks import make_identity

    f32 = mybir.dt.float32
    f32r = mybir.dt.float32r
    X = mybir.AxisListType.X
    mult = mybir.AluOpType.mult
    is_ge = mybir.AluOpType.is_ge
    Exp = mybir.ActivationFunctionType.Exp

    nc = bacc.Bacc("TRN2", target_bir_lowering=False, debug=False,
                   num_devices=NCORES)

    xs = nc.dram_tensor("xs", [TC, D], f32, kind="ExternalInput").ap()
    # fp32r-declared DRAM weights: DMA straight into fp32r SBUF tiles is
    # accepted by the BIR verifier and bit-identical to a DVE rounding pass
    # (verified empirically on HW).
    wbig = nc.dram_tensor("wbig", [KC * P, O], f32r, kind="ExternalInput").ap()
    wcat = nc.dram_tensor("wcat", [D, ER], f32r, kind="ExternalInput").ap()
    # router weights kept fp32: exact logits so top-2 selection matches the
    # fp32 reference (fp32r noise flips near-tied experts otherwise)
    wrouter = nc.dram_tensor("wrouter", [D, E], f32, kind="ExternalInput").ap()
    bias = nc.dram_tensor("bias", [P, O], f32, kind="ExternalInput").ap()
    cbias = nc.dram_tensor("cbias", [P, E], f32, kind="ExternalInput").ap()
    out = nc.dram_tensor("out", [TC, O], f32, kind="ExternalOutput").ap()

    NCAT = ER  # 256

    with tile.TileContext(nc) as tc:
        with (
            tc.tile_pool(name="persist", bufs=1) as persist,
            tc.tile_pool(name="consts", bufs=1) as consts,
        ):
            xT = persist.tile([P, DC, TC], f32r)     # x transposed, rounded
            hgT = persist.tile([P, HC, TC], f32r)    # gated h transposed
            ident = consts.tile([P, P], f32)
            make_identity(nc, ident[:])
            bias_sb = consts.tile([P, O], f32)
            nc.sync.dma_start(bias_sb[:], bias)
            cbias_sb = consts.tile([P, E], f32)
            nc.sync.dma_start(cbias_sb[:], cbias)
            wrouter_sb = consts.tile([P, DC, E], f32)
            nc.sync.dma_start(
                wrouter_sb[:], wrouter.rearrange("(kc p) n -> p kc n", p=P))
            negbig = consts.tile([P, E], f32)
            nc.vector.memset(negbig[:], -1e30)

            # PE warm-up: ~5us of dense matmuls so the HAM clock gate opens
            # (K=8/8, 2.4GHz) before phase 1 instead of 260us into the run.
            with (
                tc.tile_pool(name="wu_pool", bufs=1) as wupl,
                tc.tile_pool(name="wu_psum", bufs=1, space="PSUM") as wup,
            ):
                wu = wupl.tile([P, 512], f32)
                nc.vector.memset(wu[:], 0.0)
                wups = wup.tile([P, 512], f32)
                for _ in range(12):
                    nc.tensor.matmul(wups[:], wu[:, 0:P], wu[:],
                                     start=True, stop=True)

            # ---------------- Phase 1: transpose x, router, gates, hgT ----
            with (
                tc.tile_pool(name="wcat_pool", bufs=1) as wcat_pool,
                tc.tile_pool(name="p1", bufs=3) as p1,
                tc.tile_pool(name="x32_pool", bufs=2) as x32p,
                tc.tile_pool(name="gates_pool", bufs=2) as gp,
                tc.tile_pool(name="ph_psum", bufs=2, space="PSUM") as php,
                tc.tile_pool(name="pr_psum", bufs=2, space="PSUM") as prp,
                tc.tile_pool(name="tr_psum", bufs=3, space="PSUM") as ptp,
            ):
                wcat_sb = wcat_pool.tile([P, DC, NCAT], f32r)
                nc.sync.dma_start(
                    wcat_sb[:], wcat.rearrange("(kc p) n -> p kc n", p=P))

                for tt in range(TT):
                    ts = slice(tt * P, (tt + 1) * P)
                    # transpose x tile [128, 4096] -> xT[:, :, ts].
                    # Each transposed chunk is copied twice: rounded fp32r
                    # into persistent xT, and exact fp32 into a transient
                    # chunk that immediately feeds the router accumulation
                    # (the router must see unrounded x to match the fp32
                    # reference's top-2 selection on near-tied experts).
                    pr = prp.tile([P, E], f32, tag="pr")
                    for dc4 in range(4):
                        xc = p1.tile([P, 1024], f32, tag="xc")
                        nc.sync.dma_start(
                            xc[:], xs[ts, dc4 * 1024:(dc4 + 1) * 1024])
                        # 4 transposes share one PSUM bank (2 rounds of 4 per
                        # 1024-wide x chunk) so the psum->SBUF copies are one
                        # wide CAST + one wide COPY instead of 8 narrow ones.
                        for half in range(2):
                            pt = ptp.tile([P, 4, P], f32, tag="pt")
                            for j in range(4):
                                nc.tensor.transpose(
                                    pt[:, j, :],
                                    xc[:, (half * 4 + j) * P:
                                       (half * 4 + j + 1) * P],
                                    ident[:])
                            kc0 = dc4 * 8 + half * 4
                            nc.vector.tensor_copy(
                                xT[:, kc0:kc0 + 4, ts], pt[:])
                            x32 = x32p.tile([P, 4, P], f32, tag="x32")
                            nc.vector.tensor_copy(x32[:], pt[:])
                            for j in range(4):
                                kc = kc0 + j
                                nc.tensor.matmul(pr[:], x32[:, j, :],
                                                 wrouter_sb[:, kc, :],
                                                 start=(kc == 0),
                                                 stop=(kc == DC - 1))
                    # loraA: psum_h[t, 256] = sum_k xT.T @ wcat  (fp32r)
                    ph = php.tile([P, NCAT], f32, tag="ph")
                    for kc in range(DC):
                        nc.tensor.matmul(ph[:], xT[:, kc, ts],
                                         wcat_sb[:, kc, :],
                                         start=(kc == 0), stop=(kc == DC - 1))
                    lg_sb = gp.tile([P, E], f32, tag="lgsb")
                    nc.vector.tensor_add(lg_sb[:], pr[:], cbias_sb[:])
                    lg = lg_sb[:]
                    # top-2 renormalized softmax gates (x SCALING)
                    m1 = gp.tile([P, 1], f32, tag="m1")
                    nc.vector.reduce_max(m1[:], lg, axis=X)
                    negm1 = gp.tile([P, 1], f32, tag="negm1")
                    nc.scalar.mul(negm1[:], m1[:], -1.0)
                    e_sb = gp.tile([P, E], f32, tag="esb")
                    nc.scalar.activation(e_sb[:], lg, Exp, bias=negm1[:])
                    t1 = gp.tile([P, E], f32, tag="t1")
                    nc.vector.scalar_tensor_tensor(
                        t1[:], lg, m1[:], negbig[:], is_ge, mult)
                    masked = gp.tile([P, E], f32, tag="masked")
                    nc.vector.tensor_add(masked[:], lg, t1[:])
                    m2 = gp.tile([P, 1], f32, tag="m2")
                    nc.vector.reduce_max(m2[:], masked[:], axis=X)
                    g_sb = gp.tile([P, E], f32, tag="gsb")
                    dsum = gp.tile([P, 1], f32, tag="dsum")
                    nc.vector.scalar_tensor_tensor(
                        g_sb[:], lg, m2[:], e_sb[:], is_ge, mult,
                        accum_out=dsum[:])
                    dhalf = gp.tile([P, 1], f32, tag="dhalf")
                    nc.scalar.mul(dhalf[:], dsum[:], 1.0 / SCALING)
                    rinv = gp.tile([P, 1], f32, tag="rinv")
                    nc.vector.reciprocal(rinv[:], dhalf[:])
                    gates = gp.tile([P, E], f32, tag="gates")
                    nc.vector.tensor_scalar_mul(gates[:], g_sb[:], rinv[:])
                    # hg = h * gates (broadcast over r), straight from PSUM
                    hg = gp.tile([P, ER], f32, tag="hg")
                    nc.vector.tensor_tensor(
                        hg[:].rearrange("p (e r) -> p e r", e=E),
                        ph[:].rearrange("p (e r) -> p e r", e=E),
                        gates[:, :, None].to_broadcast([P, E, R]),
                        mult)
                    for j in range(HC):
                        pt = ptp.tile([P, P], f32, tag="pt")
                        nc.tensor.transpose(
                            pt[:], hg[:, j * P:(j + 1) * P], ident[:])
                        nc.vector.tensor_copy(hgT[:, j, ts], pt[:])

            # ---------------- Phase 2: fused [xT; hgT] @ wbig + bias ------
            with (
                tc.tile_pool(name="wstream", bufs=4) as wst,
                tc.tile_pool(name="outp", bufs=4) as outp,
                tc.tile_pool(name="po_psum", bufs=8, space="PSUM") as pop,
            ):
                KP = KC // 2  # 17 chunk-pairs
                for ot in range(OT):
                    osl = slice(ot * ON, (ot + 1) * ON)
                    ptiles = [pop.tile([P, ON], f32, tag="po",
                                       name=f"po_{ot}_{tt}")
                              for tt in range(TT)]
                    for kp in range(KP):
                        wt = wst.tile([P, 2, ON], f32r, tag="w32")
                        nc.sync.dma_start(
                            wt[:],
                            wbig[kp * 2 * P:(kp + 1) * 2 * P, osl]
                            .rearrange("(c p) n -> p c n", p=P))
                        for c in range(2):
                            kc = kp * 2 + c
                            for tt in range(TT):
                                ts = slice(tt * P, (tt + 1) * P)
                                lhsT = (xT[:, kc, ts] if kc < DC
                                        else hgT[:, kc - DC, ts])
                                nc.tensor.matmul(
                                    ptiles[tt][:], lhsT, wt[:, c, :],
                                    start=(kc == 0), stop=(kc == KC - 1))
                    for tt in range(TT):
                        ts = slice(tt * P, (tt + 1) * P)
                        osb = outp.tile([P, ON], f32, tag="osb")
                        nc.vector.tensor_add(
                            osb[:], ptiles[tt][:], bias_sb[:, osl])
                        nc.sync.dma_start(out[ts, osl], osb[:])

    nc.compile()
    return nc


def _get_compiled():
    global _compiled
    if _compiled is None:
        _compiled = _build()
    return _compiled


def kernel(**inputs):
    global LAST_RESULT
    from concourse.bass_utils import run_bass_kernel_spmd

    x = np.ascontiguousarray(np.asarray(inputs["x"], dtype=np.float32))
    base_w = np.asarray(inputs["base_w"], dtype=np.float32)
    base_b = np.asarray(inputs["base_b"], dtype=np.float32)
    router_w = np.asarray(inputs["router_w"], dtype=np.float32)
    router_b = np.asarray(inputs["router_b"], dtype=np.float32)
    lora_a = np.asarray(inputs["lora_a"], dtype=np.float32)
    lora_b = np.asarray(inputs["lora_b"], dtype=np.float32)
    top_k = int(np.asarray(inputs.get("top_k", 2)))
    assert top_k == 2, "kernel is specialized for top_k=2"

    xt = x.reshape(T, D)
    # wbig rows: base_w.T (4096) then W2 (256) with W2[e*R+r, o] = lora_b[e,o,r]
    w2 = np.ascontiguousarray(
        lora_b.transpose(0, 2, 1).reshape(ER, O).astype(np.float32))
    wbig = np.ascontiguousarray(
        np.concatenate([base_w.T, w2], axis=0).astype(np.float32))
    # wcat: A_cat columns [d, er]; router weights separate (fp32-exact path)
    acat = lora_a.reshape(ER, D)  # [er, d]
    wcat = np.ascontiguousarray(acat.T.astype(np.float32))
    wrouter = np.ascontiguousarray(router_w.T.astype(np.float32))
    bias_full = np.ascontiguousarray(
        np.broadcast_to(base_b, (P, O)).astype(np.float32))
    cbias_full = np.ascontiguousarray(
        np.broadcast_to(router_b.astype(np.float32), (P, E)))

    nc = _get_compiled()
    in_maps = [
        {"xs": np.ascontiguousarray(xt[c * TC:(c + 1) * TC]),
         "wbig": wbig, "wcat": wcat, "wrouter": wrouter,
         "bias": bias_full, "cbias": cbias_full}
        for c in range(NCORES)
    ]
    res = run_bass_kernel_spmd(nc, in_maps, core_ids=list(range(NCORES)),
                               trace=TRACE)
    LAST_RESULT = res
    outp = np.concatenate(
        [res.results[c]["out"] for c in range(NCORES)], axis=0)
    return outp.reshape(B, S, O).astype(np.float32)



# revision 10
# speedup vs baseline: 1.0520x; 1.0520x over previous
"""MoE-routing LoRA linear for Trainium2, SPMD over 8 NeuronCores.

out = x @ base_w.T + base_b + 2.0 * lora_out, where lora_out routes each
token through its top-2 (of 8) LoRA experts with renormalized softmax gates.

Strategy: data-parallel over tokens (1024 tokens/core), weights replicated.
All heavy FLOPs are fp32r matmuls (full PE rate). The per-expert LoRA is
algebraically dense: h = x @ A_cat.T ([T,256]); gated hg = h * gates[e];
lora_out = hg @ W2 ([256,4096]) which is fused into the base matmul as two
extra contraction chunks.

v2 (trace-driven): phase 1 was 212us for ~60us of PE work (HAM oscillation
from per-tile PE stalls on the softmax chain, 44us of duplicate fp32 x
copies, DMA-paced transposes). Fixes: router reads the fp32r xT buffer
through an fp32 bitcast view (fp32r SBUF bytes are raw fp32 bits; rounding
happens in the PE datapath) so the top-2 selection stays bit-exact without
a second copy; hgT transposes deferred to the end of phase 1 so the PE
never waits on the per-tile gate chain; router+loraA matmuls interleaved
per chunk; 1MB x DMAs; phase-2 weight stream pool opened before phase 1 so
its first tiles prefetch early.
"""

import numpy as np

P = 128
B, S, D, O, E, R = 4, 2048, 4096, 4096, 8, 32
T = B * S            # 8192 tokens
NCORES = 8
TC = T // NCORES     # 1024 tokens per core
TT = TC // P         # 8 token tiles per core
DC = D // P          # 32 contraction chunks for x
ER = E * R           # 256
HC = ER // P         # 2 contraction chunks for hg
KC = DC + HC         # 34 total contraction chunks in the fused matmul
ON = 512             # output tile width
OT = O // ON         # 8 output tiles
SCALING = 64.0 / 32.0

TRACE = False        # test harness sets kernel.TRACE = True for profiling
LAST_RESULT = None   # BassKernelResults of the last run (for exec_time_ns)

_compiled = None


def _build():
    import concourse.mybir as mybir
    import concourse.tile as tile
    from concourse import bacc
    from concourse.masks import make_identity

    f32 = mybir.dt.float32
    f32r = mybir.dt.float32r
    bf16 = mybir.dt.bfloat16
    X = mybir.AxisListType.X
    mult = mybir.AluOpType.mult
    is_ge = mybir.AluOpType.is_ge
    Exp = mybir.ActivationFunctionType.Exp

    nc = bacc.Bacc("TRN2", target_bir_lowering=False, debug=False,
                   num_devices=NCORES)

    xs = nc.dram_tensor("xs", [TC, D], f32, kind="ExternalInput").ap()
    # fp32r-declared DRAM weights: DMA straight into fp32r SBUF tiles is
    # accepted by the BIR verifier and bit-identical to a DVE rounding pass
    # (verified empirically on HW).
    wbig = nc.dram_tensor("wbig", [KC * P, O], f32r, kind="ExternalInput").ap()
    wcat = nc.dram_tensor("wcat", [D, ER], f32r, kind="ExternalInput").ap()
    # router weights kept fp32: exact logits so top-2 selection matches the
    # fp32 reference (fp32r noise flips near-tied experts otherwise)
    wrouter = nc.dram_tensor("wrouter", [D, E], f32, kind="ExternalInput").ap()
    bias = nc.dram_tensor("bias", [P, O], bf16, kind="ExternalInput").ap()
    cbias = nc.dram_tensor("cbias", [P, E], f32, kind="ExternalInput").ap()
    out = nc.dram_tensor("out", [TC, O], f32, kind="ExternalOutput").ap()

    NCAT = ER  # 256

    with tile.TileContext(nc) as tc:
        with (
            tc.tile_pool(name="persist", bufs=1) as persist,
            tc.tile_pool(name="consts", bufs=1) as consts,
            tc.tile_pool(name="wstream", bufs=3) as wst,
        ):
            xT = persist.tile([P, DC, TC], f32r)     # x transposed (raw fp32
            # bits under an fp32r tag: phase-2/loraA read it as fp32r, the
            # router reads the identical bytes through a fp32 bitcast view)
            hgT = persist.tile([P, HC, TC], f32r)    # gated h transposed
            ident = consts.tile([P, P], f32)
            make_identity(nc, ident[:])
            identb = consts.tile([P, P], bf16)
            nc.vector.tensor_copy(identb[:], ident[:])
            bias_sb = consts.tile([P, O], bf16)
            nc.sync.dma_start(bias_sb[:], bias)
            cbias_sb = consts.tile([P, E], f32)
            nc.sync.dma_start(cbias_sb[:], cbias)
            wrouter_sb = consts.tile([P, DC, E], f32)
            nc.sync.dma_start(
                wrouter_sb[:], wrouter.rearrange("(kc p) n -> p kc n", p=P))
            negbig = consts.tile([P, E], f32)
            nc.vector.memset(negbig[:], -1e30)

            # PE warm-up: ~2.5us of dense matmuls so the HAM clock gate opens
            # (K=8/8, 2.4GHz) at the start of phase 1.
            with (
                tc.tile_pool(name="wu_pool", bufs=1) as wupl,
                tc.tile_pool(name="wu_psum", bufs=1, space="PSUM") as wup,
            ):
                wu = wupl.tile([P, 512], f32)
                nc.vector.memset(wu[:], 0.0)
                wups = wup.tile([P, 512], f32)
                for _ in range(12):
                    nc.tensor.matmul(wups[:], wu[:, 0:P], wu[:],
                                     start=True, stop=True)

            # ---------------- Phase 1: transpose x, router, gates ----------
            with (
                tc.tile_pool(name="wcat_pool", bufs=1) as wcat_pool,
                tc.tile_pool(name="xc_pool", bufs=3) as xcp,
                tc.tile_pool(name="hg_pool", bufs=8) as hgp,
                tc.tile_pool(name="gates_pool", bufs=2) as gp,
                tc.tile_pool(name="ph_psum", bufs=2, space="PSUM") as php,
                tc.tile_pool(name="pr_psum", bufs=2, space="PSUM") as prp,
                tc.tile_pool(name="tr_psum", bufs=3, space="PSUM") as ptp,
            ):
                wcat_sb = wcat_pool.tile([P, DC, NCAT], f32r)
                nc.sync.dma_start(
                    wcat_sb[:], wcat.rearrange("(kc p) n -> p kc n", p=P))

                hgs = []
                for tt in range(TT):
                    ts = slice(tt * P, (tt + 1) * P)
                    # transpose x tile [128, 4096] -> xT[:, :, ts] in groups
                    # of 4 sharing one PSUM bank so each psum->SBUF drain is
                    # one wide CAST.
                    for q in range(4):
                        xc = xcp.tile([P, 1024], f32, tag="xc")
                        nc.sync.dma_start(
                            xc[:], xs[ts, q * 1024:(q + 1) * 1024])
                        for g in range(2):
                            pt = ptp.tile([P, 4, P], f32, tag="pt")
                            for j in range(4):
                                nc.tensor.transpose(
                                    pt[:, j, :],
                                    xc[:, (g * 4 + j) * P:(g * 4 + j + 1) * P],
                                    ident[:])
                            kc0 = q * 8 + g * 4
                            nc.vector.tensor_copy(
                                xT[:, kc0:kc0 + 4, ts], pt[:])
                    # router (exact fp32 via bitcast view of xT) and loraA
                    # (fp32r) interleaved per chunk: the 256-wide loraA
                    # matmuls cover the router LDWEIGHTS latency.
                    pr = prp.tile([P, E], f32, tag="pr")
                    ph = php.tile([P, NCAT], f32, tag="ph")
                    for kc in range(DC):
                        lhs = xT[:, kc, ts]
                        nc.tensor.matmul(ph[:], lhs, wcat_sb[:, kc, :],
                                         start=(kc == 0), stop=(kc == DC - 1))
                        nc.tensor.matmul(pr[:], lhs.bitcast(f32),
                                         wrouter_sb[:, kc, :],
                                         start=(kc == 0), stop=(kc == DC - 1))
                    lg_sb = gp.tile([P, E], f32, tag="lgsb")
                    nc.vector.tensor_add(lg_sb[:], pr[:], cbias_sb[:])
                    lg = lg_sb[:]
                    # top-2 renormalized softmax gates (x SCALING)
                    m1 = gp.tile([P, 1], f32, tag="m1")
                    nc.vector.reduce_max(m1[:], lg, axis=X)
                    negm1 = gp.tile([P, 1], f32, tag="negm1")
                    nc.scalar.mul(negm1[:], m1[:], -1.0)
                    e_sb = gp.tile([P, E], f32, tag="esb")
                    nc.scalar.activation(e_sb[:], lg, Exp, bias=negm1[:])
                    t1 = gp.tile([P, E], f32, tag="t1")
                    nc.vector.scalar_tensor_tensor(
                        t1[:], lg, m1[:], negbig[:], is_ge, mult)
                    masked = gp.tile([P, E], f32, tag="masked")
                    nc.vector.tensor_add(masked[:], lg, t1[:])
                    m2 = gp.tile([P, 1], f32, tag="m2")
                    nc.vector.reduce_max(m2[:], masked[:], axis=X)
                    g_sb = gp.tile([P, E], f32, tag="gsb")
                    dsum = gp.tile([P, 1], f32, tag="dsum")
                    nc.vector.scalar_tensor_tensor(
                        g_sb[:], lg, m2[:], e_sb[:], is_ge, mult,
                        accum_out=dsum[:])
                    dhalf = gp.tile([P, 1], f32, tag="dhalf")
                    nc.scalar.mul(dhalf[:], dsum[:], 1.0 / SCALING)
                    rinv = gp.tile([P, 1], f32, tag="rinv")
                    nc.vector.reciprocal(rinv[:], dhalf[:])
                    gates = gp.tile([P, E], f32, tag="gates")
                    nc.vector.tensor_scalar_mul(gates[:], g_sb[:], rinv[:])
                    # hg = h * gates (broadcast over r), straight from PSUM;
                    # buffered per tile so the transposes can run later
                    # without stalling the PE on this DVE chain.
                    # bf16 hg: fits SBUF, transposes at 1c/r; quantization
                    # only touches the small LoRA term (~7e-4 rel).
                    hg = hgp.tile([P, ER], bf16, tag="hg")
                    nc.vector.tensor_tensor(
                        hg[:].rearrange("p (e r) -> p e r", e=E),
                        ph[:].rearrange("p (e r) -> p e r", e=E),
                        gates[:, :, None].to_broadcast([P, E, R]),
                        mult)
                    hgs.append(hg)

                # deferred hgT transposes: by now every tile's gate chain is
                # done (except possibly the last), so the PE never idles.
                for tt in range(TT):
                    ts = slice(tt * P, (tt + 1) * P)
                    pt = ptp.tile([P, 2, P], bf16, tag="pt")
                    for j in range(HC):
                        nc.tensor.transpose(
                            pt[:, j, :], hgs[tt][:, j * P:(j + 1) * P],
                            identb[:])
                    nc.vector.tensor_copy(hgT[:, :, ts], pt[:])

            # ---------------- Phase 2: fused [xT; hgT] @ wbig + bias ------
            with (
                tc.tile_pool(name="outp", bufs=4) as outp,
                tc.tile_pool(name="po_psum", bufs=8, space="PSUM") as pop,
            ):
                KP = KC // 2  # 17 chunk-pairs
                for ot in range(OT):
                    osl = slice(ot * ON, (ot + 1) * ON)
                    ptiles = [pop.tile([P, ON], f32, tag="po",
                                       name=f"po_{ot}_{tt}")
                              for tt in range(TT)]
                    for kp in range(KP):
                        wt = wst.tile([P, 2, ON], f32r, tag="w32")
                        nc.sync.dma_start(
                            wt[:],
                            wbig[kp * 2 * P:(kp + 1) * 2 * P, osl]
                            .rearrange("(c p) n -> p c n", p=P))
                        for c in range(2):
                            kc = kp * 2 + c
                            for tt in range(TT):
                                ts = slice(tt * P, (tt + 1) * P)
                                lhsT = (xT[:, kc, ts] if kc < DC
                                        else hgT[:, kc - DC, ts])
                                nc.tensor.matmul(
                                    ptiles[tt][:], lhsT, wt[:, c, :],
                                    start=(kc == 0), stop=(kc == KC - 1))
                    for tt in range(TT):
                        ts = slice(tt * P, (tt + 1) * P)
                        osb = outp.tile([P, ON], f32, tag="osb")
                        nc.vector.tensor_add(
                            osb[:], ptiles[tt][:], bias_sb[:, osl])
                        nc.sync.dma_start(out[ts, osl], osb[:])

    nc.compile()
    return nc


def _get_compiled():
    global _compiled
    if _compiled is None:
        _compiled = _build()
    return _compiled


def kernel(**inputs):
    global LAST_RESULT
    from concourse.bass_utils import run_bass_kernel_spmd

    import ml_dtypes

    x = np.ascontiguousarray(np.asarray(inputs["x"], dtype=np.float32))
    base_w = np.asarray(inputs["base_w"], dtype=np.float32)
    base_b = np.asarray(inputs["base_b"], dtype=np.float32)
    router_w = np.asarray(inputs["router_w"], dtype=np.float32)
    router_b = np.asarray(inputs["router_b"], dtype=np.float32)
    lora_a = np.asarray(inputs["lora_a"], dtype=np.float32)
    lora_b = np.asarray(inputs["lora_b"], dtype=np.float32)
    top_k = int(np.asarray(inputs.get("top_k", 2)))
    assert top_k == 2, "kernel is specialized for top_k=2"

    xt = x.reshape(T, D)
    # wbig rows: base_w.T (4096) then W2 (256) with W2[e*R+r, o] = lora_b[e,o,r]
    w2 = np.ascontiguousarray(
        lora_b.transpose(0, 2, 1).reshape(ER, O).astype(np.float32))
    wbig = np.ascontiguousarray(
        np.concatenate([base_w.T, w2], axis=0).astype(np.float32))
    # wcat: A_cat columns [d, er]; router weights separate (fp32-exact path)
    acat = lora_a.reshape(ER, D)  # [er, d]
    wcat = np.ascontiguousarray(acat.T.astype(np.float32))
    wrouter = np.ascontiguousarray(router_w.T.astype(np.float32))
    bias_full = np.ascontiguousarray(
        np.broadcast_to(base_b, (P, O)).astype(ml_dtypes.bfloat16))
    cbias_full = np.ascontiguousarray(
        np.broadcast_to(router_b.astype(np.float32), (P, E)))

    nc = _get_compiled()
    in_maps = [
        {"xs": np.ascontiguousarray(xt[c * TC:(c + 1) * TC]),
         "wbig": wbig, "wcat": wcat, "wrouter": wrouter,
         "bias": bias_full, "cbias": cbias_full}
        for c in range(NCORES)
    ]
    res = run_bass_kernel_spmd(nc, in_maps, core_ids=list(range(NCORES)),
                               trace=TRACE)
    LAST_RESULT = res
    outp = np.concatenate(
        [res.results[c]["out"] for c in range(NCORES)], axis=0)
    return outp.reshape(B, S, O).astype(np.float32)
